# revision 11
# baseline (speedup 1.0000x reference)
"""Trainium2 Bass kernel for nn_MemoryAugmentedCNN (scatter_memory).

Strategy: data-parallel over batch across 8 NeuronCores (256 batch each).
Convs are M-packed matmuls (output row-classes folded into the PE M dim via
host-packed zero-padded weights); activations live in SBUF as
[row-class*channel, (y, x, b)] with batch innermost so every conv rhs is a
contiguous 512-slice.  BatchNorm uses global batch stats via one tiny
AllGather per layer (bn_stats/bn_aggr on-device, class-fold via an
indicator matmul).  All matmuls in bf16 (fp32 PSUM accumulate); BN and
softmax math in fp32.  BN+GELU applied in-place fused on ScalarE.

Host-side prep in numpy: batch sharding, bf16 casts, conv-weight packing,
im2col of x, W_head/W_out row permutation matching the on-chip hiddenT
layout.
"""
import os
import sys
import numpy as np

sys.path.insert(0, "/opt/trn_rl_repo")

import ml_dtypes  # noqa: E402
import concourse.tile as tile  # noqa: E402
from concourse import bacc, mybir  # noqa: E402
from concourse.bass_utils import run_bass_kernel_spmd  # noqa: E402

BF = ml_dtypes.bfloat16
F32, BF16 = mybir.dt.float32, mybir.dt.bfloat16
AF = mybir.ActivationFunctionType
AX = mybir.AxisListType

N_CORES = 8
B = 2048
BS = B // N_CORES          # 256 batch per core
NC, ME, CE = 1000, 512, 4096
EPS = 1e-5

H1 = 12                    # spatial after conv1 (then 10, then 8)
F1 = 3 * 12 * BS           # 9216;  h1 partition (y%4)*32+c, free (y//4, x, b)
F2 = 5 * 10 * BS           # 12800; h2 partition (y%2)*64+c, free (y//2, x, b)
F3 = 4 * 8 * BS            # 8192;  h3 partition (y%2)*64+c, free (y//2, x, b)
NT1, NT2, NT3 = F1 // 512, F2 // 512, F3 // 512  # 18, 25, 16

_COMPILED = None


# ---------------------------------------------------------------- host prep
def _pack_conv1_w(w1):
    out = np.zeros((54, 128), np.float32)
    for r6 in range(6):
        for dx in range(3):
            for ci in range(3):
                k = (r6 * 3 + dx) * 3 + ci
                for yr in range(4):
                    dy = r6 - yr
                    if 0 <= dy <= 2:
                        out[k, yr * 32:(yr + 1) * 32] = w1[:, ci, dy, dx]
    return out.astype(BF)


def _pack_conv2_w(w2):
    # we: even-yq2 K=128 blocks.  wab: odd-yq2; partitions 64:128 hold the
    # A block (input rows 2yq2+{0,1}, h1 classes 2,3), partitions 0:64 the
    # B block (rows 2yq2+{2,3}, classes 0,1) so lhsT/rhs base partitions match.
    we = np.zeros((3, 128, 128), np.float32)
    wab = np.zeros((3, 128, 128), np.float32)
    for dx in range(3):
        for ci in range(32):
            for yr in range(2):
                for j in range(4):
                    dy = j - yr
                    if 0 <= dy <= 2:
                        we[dx, j * 32 + ci, yr * 64:(yr + 1) * 64] = w2[:, ci, dy, dx]
                for j in range(2):
                    dy = j - yr
                    if 0 <= dy <= 2:
                        wab[dx, 64 + j * 32 + ci, yr * 64:(yr + 1) * 64] = w2[:, ci, dy, dx]
                for j in range(2):
                    dy = j + 2 - yr
                    if 0 <= dy <= 2:
                        wab[dx, j * 32 + ci, yr * 64:(yr + 1) * 64] = w2[:, ci, dy, dx]
    return we.astype(BF), wab.astype(BF)


def _pack_conv3_w(w3):
    out = np.zeros((2, 3, 128, 128), np.float32)
    for blk in range(2):
        for dx in range(3):
            for j2 in range(2):
                for ci in range(64):
                    for yr in range(2):
                        dy = 2 * blk + j2 - yr
                        if 0 <= dy <= 2:
                            out[blk, dx, j2 * 64 + ci, yr * 64:(yr + 1) * 64] = \
                                w3[:, ci, dy, dx]
    return out.reshape(6, 128, 128).astype(BF)


def _hidden_perm():
    # original hidden index k = c*64 + y*8 + x  ->  my hiddenT position
    # pos = kt*128 + pi*64 + c2*32 + yq*8 + x, c = 2kt+c2, y = 2yq+pi
    k = np.arange(CE)
    c, y, x = k // 64, (k % 64) // 8, k % 8
    return (c // 2) * 128 + (y % 2) * 64 + (c % 2) * 32 + (y // 2) * 8 + x


def _im2col_x(xs):
    # xs [BS, 3, 14, 14] f32 -> [54, 3*12*BS] bf16, rows (r6*3+dx)*3+ci
    out = np.empty((54, 3, H1, BS), np.float32)
    for r6 in range(6):
        for dx in range(3):
            for ci in range(3):
                k = (r6 * 3 + dx) * 3 + ci
                for yq in range(3):
                    out[k, yq] = xs[:, ci, 4 * yq + r6, dx:dx + H1].T
    return np.ascontiguousarray(out.reshape(54, F1)).astype(BF)


def _prep(inputs):
    x = np.asarray(inputs["x"], np.float32)
    w1p = _pack_conv1_w(np.asarray(inputs["w1"], np.float32))
    w2pe, w2pab = _pack_conv2_w(np.asarray(inputs["w2"], np.float32))
    w3p = _pack_conv3_w(np.asarray(inputs["w3"], np.float32))

    perm = _hidden_perm()
    Whp = np.empty((CE, NC), np.float32)
    Whp[perm] = np.asarray(inputs["W_head"], np.float32)
    Wop = np.empty((CE + ME, NC), np.float32)
    Wo_in = np.asarray(inputs["W_out"], np.float32)
    Wop[perm] = Wo_in[:CE]
    Wop[CE:] = Wo_in[CE:]

    def rep(v, t):
        return np.tile(np.asarray(v, np.float32), t)[:, None]

    gb1 = np.concatenate([rep(inputs["g1"], 4), rep(inputs["be1"], 4)], axis=1)
    gb2 = np.concatenate([rep(inputs["g2"], 2), rep(inputs["be2"], 2)], axis=1)
    gb3 = np.concatenate([rep(inputs["g3"], 2), rep(inputs["be3"], 2)], axis=1)

    p = np.arange(128)
    m4 = (p[:, None] % 32 == p[None, :] % 32).astype(np.float32)
    m2 = (p[:, None] % 64 == p[None, :] % 64).astype(np.float32)

    shared = {
        "w1p": w1p, "w2pe": w2pe, "w2pab": w2pab, "w3p": w3p,
        "wh": Whp.astype(BF), "wo": Wop.astype(BF),
        "mem": np.asarray(inputs["memory"], np.float32).astype(BF),
        "bh": np.asarray(inputs["b_head"], np.float32)[None, :].astype(BF),
        "bo": np.asarray(inputs["b_out"], np.float32)[None, :].astype(BF),
        "ones1": np.ones((1, 128), BF),
        "ident": np.eye(128, dtype=BF),
        "gb1": gb1, "gb2": gb2, "gb3": gb3, "m4": m4, "m2": m2,
        "epsc": np.full((128, 1), EPS, np.float32),
    }
    xims = [_im2col_x(x[c * BS:(c + 1) * BS]) for c in range(N_CORES)]
    return shared, xims


# ---------------------------------------------------------------- device
def _build():
    nc = bacc.Bacc("TRN2", target_bir_lowering=False, debug=False,
                   num_devices=N_CORES)

    def din(name, shape, dt=BF16):
        return nc.dram_tensor(name, shape, dt, kind="ExternalInput").ap()

    d = dict(
        xim=din("xim", [54, F1]),
        w1p=din("w1p", [54, 128]),
        w2pe=din("w2pe", [3, 128, 128]),
        w2pab=din("w2pab", [3, 128, 128]),
        w3p=din("w3p", [6, 128, 128]),
        wh=din("wh", [CE, NC]),
        wo=din("wo", [CE + ME, NC]),
        mem=din("mem", [NC, ME]),
        bh=din("bh", [1, NC]),
        bo=din("bo", [1, NC]),
        ones1=din("ones1", [1, 128]),
        ident=din("ident", [128, 128]),
        gb1=din("gb1", [128, 2], F32),
        gb2=din("gb2", [128, 2], F32),
        gb3=din("gb3", [128, 2], F32),
        m4=din("m4", [128, 128], F32),
        m2=din("m2", [128, 128], F32),
        epsc=din("epsc", [128, 1], F32),
        out_d=nc.dram_tensor("out", [BS, NC], F32, kind="ExternalOutput").ap(),
        ag_in=[nc.dram_tensor(f"ag_in{i}", [128, 2], F32) for i in range(3)],
        ag_out=[nc.dram_tensor(f"ag_out{i}", [128 * N_CORES, 2], F32,
                               addr_space="Shared") for i in range(3)],
    )
    with tile.TileContext(nc) as tc:
        _emit(nc, tc, d)
    nc.compile()
    return nc


def _emit(nc, tc, d):
    STAGE = int(os.environ.get("KSTAGE", "99"))
    from contextlib import ExitStack
    es = ExitStack()
    consts = es.enter_context(tc.tile_pool(name="consts", bufs=1))
    acts = es.enter_context(tc.tile_pool(name="acts", bufs=1))
    wpool = es.enter_context(tc.tile_pool(name="wpool", bufs=1))
    wring = es.enter_context(tc.tile_pool(name="wring", bufs=16))
    small = es.enter_context(tc.tile_pool(name="small", bufs=1))

    # ---- consts ----
    def cload(name, shape, src, dt=BF16):
        t = consts.tile(shape, dt, tag=name)
        nc.sync.dma_start(t[:], src)
        return t

    W1 = cload("W1", [54, 128], d["w1p"][:])
    W2E = consts.tile([128, 3 * 128], BF16, tag="W2E")
    nc.sync.dma_start(W2E[:].rearrange("p (k m) -> p k m", k=3),
                      d["w2pe"][:].rearrange("k p m -> p k m"))
    W2AB = consts.tile([128, 3 * 128], BF16, tag="W2AB")
    nc.sync.dma_start(W2AB[:].rearrange("p (k m) -> p k m", k=3),
                      d["w2pab"][:].rearrange("k p m -> p k m"))
    W3 = consts.tile([128, 6 * 128], BF16, tag="W3")
    nc.sync.dma_start(W3[:].rearrange("p (k m) -> p k m", k=6),
                      d["w3p"][:].rearrange("k p m -> p k m"))
    ONES = cload("ONES", [1, 128], d["ones1"][:])
    IDENT = cload("IDENT", [128, 128], d["ident"][:])
    BH = cload("BH", [1, NC], d["bh"][:])
    BO = cload("BO", [1, NC], d["bo"][:])
    M4T = cload("M4T", [128, 128], d["m4"][:], F32)
    M2T = cload("M2T", [128, 128], d["m2"][:], F32)
    EPSC = cload("EPSC", [128, 1], d["epsc"][:], F32)
    GB = [cload(f"GB{i}", [128, 2], d[f"gb{i+1}"][:], F32) for i in range(3)]

    # ---- weight prefetch: W_head fully (split per-kt for DMA parallelism),
    #      memory fully; W_out streamed through a 16-deep ring ----
    WH = wpool.tile([128, 32 * NC], BF16, tag="WH")
    for kt in range(32):
        nc.sync.dma_start(
            WH[:, kt * NC:(kt + 1) * NC],
            d["wh"][kt * 128:(kt + 1) * 128, :].rearrange("(o p) n -> p (o n)", o=1))
    MEM = wpool.tile([125, 8 * ME], BF16, tag="MEM")
    for cb in range(8):
        nc.sync.dma_start(
            MEM[:, cb * ME:(cb + 1) * ME],
            d["mem"][cb * 125:(cb + 1) * 125, :].rearrange("(o p) n -> p (o n)", o=1))
    WOt = []
    for kt in range(36):
        w = wring.tile([128, NC], BF16, tag="wo")
        nc.gpsimd.dma_start(
            w[:], d["wo"][kt * 128:(kt + 1) * 128, :]
            .rearrange("(o p) n -> p (o n)", o=1))
        WOt.append(w)

    # ---- BN helper ----
    def bn_layer(idx, raw, ntiles, fold_mat, inv_cnt):
        bns = small.tile([128, ntiles * 6], F32, tag=f"bns{idx}")
        for tt in range(ntiles):
            nc.vector.bn_stats(bns[:, tt * 6:(tt + 1) * 6],
                               raw[:, tt * 512:(tt + 1) * 512])
        agg = small.tile([128, 2], F32, tag=f"agg{idx}")
        nc.vector.bn_aggr(agg[:], bns[:])
        mu2 = small.tile([128, 1], F32, tag=f"mu2{idx}")
        nc.vector.tensor_mul(mu2[:], agg[:, 0:1], agg[:, 0:1])
        nc.vector.tensor_add(agg[:, 1:2], agg[:, 1:2], mu2[:])
        nc.gpsimd.dma_start(d["ag_in"][idx][:], agg[:])
        nc.gpsimd.collective_compute(
            "AllGather", mybir.AluOpType.bypass,
            replica_groups=[list(range(N_CORES))],
            ins=[d["ag_in"][idx][:].opt()], outs=[d["ag_out"][idx][:].opt()])
        gat = small.tile([128, 2 * N_CORES], F32, tag=f"gat{idx}")
        nc.gpsimd.dma_start(
            gat[:].rearrange("p (s c) -> p s c", c=N_CORES),
            d["ag_out"][idx][:].rearrange("(c p) s -> p s c", c=N_CORES))
        csum = small.tile([128, 2], F32, tag=f"csum{idx}")
        nc.vector.reduce_sum(csum[:].rearrange("p (s o) -> p s o", o=1),
                             gat[:].rearrange("p (s c) -> p s c", c=N_CORES),
                             axis=AX.X)
        with tc.tile_pool(name=f"psb{idx}", bufs=1, space="PSUM") as psb:
            pstat = psb.tile([128, 2], F32, tag="pstat")
            nc.tensor.matmul(pstat[:], fold_mat[:], csum[:], start=True, stop=True)
            stats = small.tile([128, 2], F32, tag=f"stats{idx}")
            nc.scalar.activation(stats[:], pstat[:], AF.Copy, scale=inv_cnt)
        var = small.tile([128, 1], F32, tag=f"var{idx}")
        nc.vector.tensor_mul(var[:], stats[:, 0:1], stats[:, 0:1])
        nc.vector.tensor_sub(var[:], stats[:, 1:2], var[:])
        sd = small.tile([128, 1], F32, tag=f"sd{idx}")
        nc.scalar.activation(sd[:], var[:], AF.Sqrt, bias=EPSC[:])
        inv = small.tile([128, 1], F32, tag=f"inv{idx}")
        nc.vector.reciprocal(inv[:], sd[:])
        scale = small.tile([128, 1], F32, tag=f"scale{idx}")
        nc.vector.tensor_mul(scale[:], inv[:], GB[idx][:, 0:1])
        shift = small.tile([128, 1], F32, tag=f"shift{idx}")
        nc.vector.tensor_mul(shift[:], stats[:, 0:1], scale[:])
        nc.vector.tensor_sub(shift[:], GB[idx][:, 1:2], shift[:])
        return scale, shift

    def dump_and_stop(src_tile):
        p = src_tile.shape[0]
        w = min(src_tile.shape[-1] if len(src_tile.shape) == 2 else NC, NC)
        with tc.tile_pool(name="dmp", bufs=1) as dp:
            oe = dp.tile([128, NC], F32, tag="dmp")
            nc.gpsimd.memset(oe[:], 0.0)
            nc.scalar.copy(oe[0:p, 0:w], src_tile[0:p, 0:w])
            nc.sync.dma_start(d["out_d"][0:128, :], oe[:])
            z = dp.tile([128, NC], F32, tag="dmpz")
            nc.gpsimd.memset(z[:], 0.0)
            nc.sync.dma_start(d["out_d"][128:256, :], z[:])
        es.close()

    def gelu_inplace(h, ntiles, scale, shift):
        for tt in range(ntiles):
            nc.scalar.activation(h[:, tt * 512:(tt + 1) * 512],
                                 h[:, tt * 512:(tt + 1) * 512],
                                 AF.Gelu, bias=shift[:], scale=scale[:])

    # ---- conv1 ----  (XIM -> slot bigA; h1 -> slot bigB)
    XIM = acts.tile([54, F1], BF16, tag="bigA")
    nc.sync.dma_start(XIM[:], d["xim"][:])
    h1 = acts.tile([128, F1], BF16, tag="bigB")
    with tc.tile_pool(name="cps1", bufs=4, space="PSUM") as cps:
        for tt in range(NT1):
            pt = cps.tile([128, 512], F32, tag="c1")
            nc.tensor.matmul(pt[:], W1[:], XIM[:, tt * 512:(tt + 1) * 512],
                             start=True, stop=True)
            if tt % 2 == 0:
                nc.scalar.copy(h1[:, tt * 512:(tt + 1) * 512], pt[:])
            else:
                nc.vector.tensor_copy(h1[:, tt * 512:(tt + 1) * 512], pt[:])
        sc1, sh1 = bn_layer(0, h1, NT1, M4T, 1.0 / 32.0)
    gelu_inplace(h1, NT1, sc1, sh1)
    if STAGE == 1:
        dump_and_stop(h1[:])
        return

    # ---- conv2 ----  (h2 -> slot bigA, reusing XIM's slot)
    h2 = acts.tile([128, F2], BF16, tag="bigA")
    with tc.tile_pool(name="cps2", bufs=6, space="PSUM") as cps, \
         tc.tile_pool(name="cs2", bufs=3) as cs2:
        for yq2 in range(5):
            for i in range(5):
                off = (yq2 * 10 + i * 2) * BS
                if yq2 % 2 == 0:
                    pt = cps.tile([128, 512], F32, tag="c2")
                    for dx in range(3):
                        base = (yq2 // 2) * 12 * BS + (2 * i + dx) * BS
                        nc.tensor.matmul(pt[:], W2E[:, dx * 128:(dx + 1) * 128],
                                         h1[:, base:base + 512],
                                         start=(dx == 0), stop=(dx == 2))
                    if (yq2 * 5 + i) % 2 == 0:
                        nc.scalar.copy(h2[:, off:off + 512], pt[:])
                    else:
                        nc.vector.tensor_copy(h2[:, off:off + 512], pt[:])
                else:
                    # mixing (0,0)- and (64,0)-positioned matmuls in one PSUM
                    # accumulation group aborts the NEFF on HW; use two
                    # groups and fuse with an add on eviction.
                    ptA = cps.tile([128, 512], F32, tag="c2")
                    ptB = cps.tile([128, 512], F32, tag="c2")
                    for dx in range(3):
                        baseA = (yq2 // 2) * 12 * BS + (2 * i + dx) * BS
                        baseB = (yq2 // 2 + 1) * 12 * BS + (2 * i + dx) * BS
                        nc.tensor.matmul(ptA[:],
                                         W2AB[64:128, dx * 128:(dx + 1) * 128],
                                         h1[64:128, baseA:baseA + 512],
                                         start=(dx == 0), stop=(dx == 2))
                        nc.tensor.matmul(ptB[:],
                                         W2AB[0:64, dx * 128:(dx + 1) * 128],
                                         h1[0:64, baseB:baseB + 512],
                                         start=(dx == 0), stop=(dx == 2))
                    sc = cs2.tile([128, 512], F32, tag="c2s")
                    nc.scalar.copy(sc[:], ptB[:])
                    nc.vector.tensor_add(h2[:, off:off + 512], ptA[:], sc[:])
        sc2, sh2 = bn_layer(1, h2, NT2, M2T, 1.0 / 16.0)
    if STAGE == 15:
        dump_and_stop(h2[:])
        return
    gelu_inplace(h2, NT2, sc2, sh2)
    if STAGE == 2:
        dump_and_stop(h2[:])
        return

    # ---- conv3 ----  (h3 -> slot bigB, reusing h1's slot)
    h3 = acts.tile([128, F3], BF16, tag="bigB")
    with tc.tile_pool(name="cps3", bufs=4, space="PSUM") as cps:
        for yq3 in range(4):
            for i in range(4):
                pt = cps.tile([128, 512], F32, tag="c3")
                for blk in range(2):
                    for dx in range(3):
                        base = (yq3 + blk) * 10 * BS + (2 * i + dx) * BS
                        nc.tensor.matmul(
                            pt[:],
                            W3[:, (blk * 3 + dx) * 128:(blk * 3 + dx + 1) * 128],
                            h2[:, base:base + 512],
                            start=(blk == 0 and dx == 0),
                            stop=(blk == 1 and dx == 2))
                off = (yq3 * 8 + i * 2) * BS
                if i % 2 == 0:
                    nc.scalar.copy(h3[:, off:off + 512], pt[:])
                else:
                    nc.vector.tensor_copy(h3[:, off:off + 512], pt[:])
        sc3, sh3 = bn_layer(2, h3, NT3, M2T, 1.0 / 16.0)
    gelu_inplace(h3, NT3, sc3, sh3)
    if STAGE == 3:
        dump_and_stop(h3[:])
        return

    # ---- hiddenT: hT[p, kt*BS+b], p = pi*64 + c2*32 + yq*8 + x ----
    hT = acts.tile([128, 32 * BS], BF16, tag="hT")
    for kt in range(32):
        for pi in range(2):
            for c2 in range(2):
                sp = pi * 64 + 2 * kt + c2
                dp = pi * 64 + c2 * 32
                nc.sync.dma_start(
                    hT[dp:dp + 32, kt * BS:(kt + 1) * BS],
                    h3[sp:sp + 1, :])

    if STAGE == 4:
        dump_and_stop(hT[:])
        return

    # ---- head GEMM + softmax + transpose ----
    rwT = acts.tile([125, 8 * BS], BF16, tag="rwT")
    with tc.tile_pool(name="kps", bufs=2, space="PSUM") as kps, \
         tc.tile_pool(name="sft", bufs=2) as sft, \
         tc.tile_pool(name="tps", bufs=4, space="PSUM") as tps:
        for bt in range(2):
            kp = kps.tile([128, 1024], F32, tag="keys")
            for kt in range(32):
                lhsT = hT[:, kt * BS + bt * 128: kt * BS + bt * 128 + 128]
                for nt in range(2):
                    nc.tensor.matmul(
                        kp[:, nt * 512: nt * 512 + 500], lhsT,
                        WH[:, kt * NC + nt * 500: kt * NC + (nt + 1) * 500],
                        start=(kt == 0), stop=False)
            for nt in range(2):
                nc.tensor.matmul(kp[:, nt * 512: nt * 512 + 500], ONES[:],
                                 BH[:, nt * 500:(nt + 1) * 500],
                                 start=False, stop=True)
            kview = kp[:].rearrange("p (t c) -> p t c", t=2)[:, :, 0:500]
            mx = sft.tile([128, 1], F32, tag="mx")
            nc.vector.reduce_max(mx[:].rearrange("p (t o) -> p t o", t=1),
                                 kview, axis=AX.XY)
            nm = sft.tile([128, 1], F32, tag="nm")
            nc.vector.tensor_scalar_mul(nm[:], mx[:], -1.0)
            rwe = sft.tile([128, NC], F32, tag="rwe")
            zz = sft.tile([128, 1], F32, tag="zz")
            nc.scalar.activation(rwe[:].rearrange("p (t c) -> p t c", t=2),
                                 kview, AF.Exp, bias=nm[:], scale=1.0,
                                 accum_out=zz[:])
            rz = sft.tile([128, 1], F32, tag="rz")
            nc.vector.reciprocal(rz[:], zz[:])
            rwt = sft.tile([128, NC], BF16, tag="rwt")
            nc.vector.tensor_scalar_mul(rwt[:], rwe[:], rz[:])
            for cb in range(8):
                tp = tps.tile([125, 128], BF16, tag="tp")
                nc.tensor.transpose(tp[:], rwt[:, cb * 125:(cb + 1) * 125],
                                    IDENT[:])
                nc.scalar.copy(rwT[:, cb * BS + bt * 128: cb * BS + (bt + 1) * 128],
                               tp[:])

    if STAGE == 5:
        dump_and_stop(rwT[:])
        return

    # ---- read GEMM: readT [me, b] ----
    rT = acts.tile([128, 4 * BS], BF16, tag="rT")
    with tc.tile_pool(name="rps", bufs=4, space="PSUM") as rps:
        for mt in range(4):
            rp = rps.tile([128, BS], F32, tag="rp")
            for cb in range(8):
                nc.tensor.matmul(
                    rp[:], MEM[:, cb * ME + mt * 128: cb * ME + (mt + 1) * 128],
                    rwT[:, cb * BS:(cb + 1) * BS],
                    start=(cb == 0), stop=(cb == 7))
            nc.scalar.copy(rT[:, mt * BS:(mt + 1) * BS], rp[:])

    if STAGE == 6:
        dump_and_stop(rT[:])
        return

    # ---- final GEMM ----
    with tc.tile_pool(name="ops", bufs=2, space="PSUM") as ops, \
         tc.tile_pool(name="oev", bufs=2) as oev:
        for bt in range(2):
            op = ops.tile([128, 1024], F32, tag="out")
            for kt in range(36):
                if kt < 32:
                    lhsT = hT[:, kt * BS + bt * 128: kt * BS + bt * 128 + 128]
                else:
                    mt = kt - 32
                    lhsT = rT[:, mt * BS + bt * 128: mt * BS + bt * 128 + 128]
                for nt in range(2):
                    nc.tensor.matmul(
                        op[:, nt * 512: nt * 512 + 500], lhsT,
                        WOt[kt][:, nt * 500:(nt + 1) * 500],
                        start=(kt == 0), stop=False)
            for nt in range(2):
                nc.tensor.matmul(op[:, nt * 512: nt * 512 + 500], ONES[:],
                                 BO[:, nt * 500:(nt + 1) * 500],
                                 start=False, stop=True)
            oe = oev.tile([128, NC], F32, tag="oe")
            nc.scalar.activation(oe[:].rearrange("p (t c) -> p t c", t=2),
                                 kp_view_out(op), AF.Copy)
            nc.sync.dma_start(d["out_d"][bt * 128:(bt + 1) * 128, :], oe[:])
    es.close()


def kp_view_out(op):
    return op[:].rearrange("p (t c) -> p t c", t=2)[:, :, 0:500]


def kernel(**inputs) -> np.ndarray:
    global _COMPILED
    if _COMPILED is None:
        _COMPILED = _build()
    nc = _COMPILED
    shared, xims = _prep(inputs)
    in_maps = [{**shared, "xim": xims[c]} for c in range(N_CORES)]
    res = run_bass_kernel_spmd(nc, in_maps, core_ids=list(range(N_CORES)))
    return np.concatenate([res.results[c]["out"] for c in range(N_CORES)],
                          axis=0)
